# revision 36
# baseline (speedup 1.0000x reference)
"""nn_ARDecoder Trainium2 Bass kernel.

Shapes (hardcoded): context [64,512,512] f32, static_embed [64,128] f32,
H=4 heads, future_steps=64, OUT=1. Output preds [64,64,1] f32.

Sharding: data-parallel over batch B=64 across 8 cores (8 batches/core),
weights replicated, GRU scan local per shard. No collectives.

Algebraic structure exploited:
  - reference only uses sum_q w_q * attn_out[b,q,:], so the full a@v einsum
    and out-projection collapse to tiny vector contractions; W_out and
    W_init compose into one matrix host-side.
  - OUT=1 makes W_tf rank-1: with b_tf==0, inp_t = relu(pred*u) =
    p+ * relu(u) + p- * (-relu(-u)), so the inp part of the GRU input
    projection is rank-2 (two precomputed 1536-vectors scaled by
    max(pred,0)/min(pred,0) per batch).
  - static_embed part of gi is step-invariant: precomputed host-side.

GRU scan performance structure (v2):
  - The three h-projection gates (r, z, hn) and the i_n combo run in four
    PE column groups concurrently (tile_position=(0,32j)), all into one
    PSUM bank at partition quadrants 0/32/64/96.  Matmul stream time per
    step ~= 5 x 213ns instead of 15 x 213ns.
  - sigmoid/tanh evacuate PSUM->SBUF as part of the activation (no
    separate copies).  t = r*h_n is the only batch-major DVE op; the
    "+ i_n" add is done on the tensor engine by accumulating t into the
    i_n PSUM group via an identity matmul.
  - The h update runs in transposed layout [512dims x 8batch] on 128
    partitions (DVE free-dim 32 instead of 512), and h stays transposed
    across steps, feeding the next step's matmuls directly as lhsT.
"""

import os
import sys
import traceback

import numpy as np

sys.path.insert(0, "/opt/trn_rl_repo")

H = 4
B, LC, C, S = 64, 512, 512, 128
T = 64  # future steps
NCORES = 8
BS = B // NCORES  # 8 batches per core

_COMPILED = None  # (nc,) cache


# ---------------------------------------------------------------- numpy ref
def _np_softmax(x, axis):
    m = np.max(x, axis=axis, keepdims=True)
    e = np.exp(x - m)
    return e / np.sum(e, axis=axis, keepdims=True)


def _np_sigmoid(x):
    out = np.empty_like(x)
    pos = x >= 0
    out[pos] = 1.0 / (1.0 + np.exp(-x[pos]))
    ex = np.exp(x[~pos])
    out[~pos] = ex / (1.0 + ex)
    return out


def _np_fallback(inputs):
    """Exact numpy replica of the reference (correctness fallback)."""
    context = np.asarray(inputs["context"], np.float32)
    static_embed = np.asarray(inputs["static_embed"], np.float32)
    W_in, b_in = inputs["W_in_attn"], inputs["b_in_attn"]
    W_out, b_out = inputs["W_out_attn"], inputs["b_out_attn"]
    W_init, b_init = inputs["W_init"], inputs["b_init"]
    W_ih, b_ih = inputs["W_ih"], inputs["b_ih"]
    W_hh, b_hh = inputs["W_hh"], inputs["b_hh"]
    W_tf, b_tf = inputs["W_tf"], inputs["b_tf"]
    W_head, b_head = inputs["W_head"], inputs["b_head"]
    fs = int(np.asarray(inputs["future_steps"]))

    Bn, L, Cn = context.shape
    D = Cn // H
    qkv = context @ W_in.T + b_in
    q, k, v = np.split(qkv, 3, axis=-1)
    heads = lambda t: t.reshape(Bn, L, H, D).transpose(0, 2, 1, 3)
    q, k, v = heads(q), heads(k), heads(v)
    scores = np.einsum("bhqd,bhkd->bhqk", q, k) / np.sqrt(np.float32(D))
    a = _np_softmax(scores, axis=-1)
    out = np.einsum("bhqk,bhkd->bhqd", a, v).transpose(0, 2, 1, 3).reshape(Bn, L, Cn)
    attn_out = out @ W_out.T + b_out
    attn_w = a.mean(axis=1)
    w = _np_softmax(attn_w.mean(axis=1), axis=-1)
    summary = np.einsum("bk,bkc->bc", w, attn_out)
    h = summary @ W_init.T + b_init
    inp = context.mean(axis=1)
    preds = np.empty((Bn, fs, W_head.shape[0]), np.float32)
    for t in range(fs):
        x = np.concatenate([inp, static_embed], axis=-1)
        gi = x @ W_ih.T + b_ih
        gh = h @ W_hh.T + b_hh
        i_r, i_z, i_n = np.split(gi, 3, axis=-1)
        h_r, h_z, h_n = np.split(gh, 3, axis=-1)
        r = _np_sigmoid(i_r + h_r)
        z = _np_sigmoid(i_z + h_z)
        n = np.tanh(i_n + r * h_n)
        h = (1.0 - z) * n + z * h
        pred = h @ W_head.T + b_head
        inp = np.maximum(pred @ W_tf.T + b_tf, 0.0)
        preds[:, t, :] = pred
    return preds


# ---------------------------------------------------------------- builder
def _build():
    import concourse.bacc as bacc
    import concourse.mybir as mybir
    import concourse.tile as tile
    from concourse import masks

    dt = mybir.dt
    AF = mybir.ActivationFunctionType
    ALU = mybir.AluOpType
    AX = mybir.AxisListType

    nc = bacc.Bacc("TRN2", target_bir_lowering=False, debug=False,
                   num_devices=NCORES)

    f32, bf16 = dt.float32, dt.bfloat16

    # DRAM I/O (per-core shard tensors; host preps layouts)
    ctxT_d = nc.dram_tensor("ctxT", [BS * 512, 512], bf16, kind="ExternalInput")
    inp0T_d = nc.dram_tensor("inp0T", [128, 4 * BS], bf16, kind="ExternalInput")
    wqk_d = nc.dram_tensor("wqk", [512, 1024], bf16, kind="ExternalInput")
    wv_d = nc.dram_tensor("wv", [512, 512], bf16, kind="ExternalInput")
    bqk_d = nc.dram_tensor("bqk", [128, 8], f32, kind="ExternalInput")
    wcomb_d = nc.dram_tensor("wcomb", [512, 512], bf16, kind="ExternalInput")
    bcomb_d = nc.dram_tensor("bcomb", [1, 512], bf16, kind="ExternalInput")
    whh_d = nc.dram_tensor("whh", [512, 1536], bf16, kind="ExternalInput")
    wihA_d = nc.dram_tensor("wihA", [512, 1536], bf16, kind="ExternalInput")
    combo_d = nc.dram_tensor("combo", [65, 1536], bf16, kind="ExternalInput")
    whead_d = nc.dram_tensor("whead", [128, 4], bf16, kind="ExternalInput")
    bhead_d = nc.dram_tensor("bhead", [1, 1], f32, kind="ExternalInput")
    preds_d = nc.dram_tensor("preds", [BS, T], f32, kind="ExternalOutput")

    ISQD = 1.0 / np.sqrt(128.0)  # 1/sqrt(D)
    IHL = 1.0 / (H * 512.0)      # 1/(H*L) for attn-weight mean
    with tile.TileContext(nc) as tc:
        with (
            tc.tile_pool(name="const", bufs=1) as cp,
            tc.tile_pool(name="state", bufs=1) as st,
        ):
            # ---- constants into SBUF
            id_bf = cp.tile([128, 128], bf16)
            masks.make_identity(nc, id_bf[:])
            id_f32 = cp.tile([128, 128], f32)
            masks.make_identity(nc, id_f32[:])
            wqk_sb = cp.tile([128, 4, 1024], bf16)
            nc.sync.dma_start(wqk_sb[:], wqk_d[:].rearrange("(c p) n -> p c n", p=128))
            wv_sb = cp.tile([128, 4, 512], bf16)
            nc.sync.dma_start(wv_sb[:], wv_d[:].rearrange("(c p) n -> p c n", p=128))
            bqk_sb = cp.tile([128, 8], f32)
            nc.sync.dma_start(bqk_sb[:], bqk_d[:])
            wcomb_sb = cp.tile([128, 4, 512], bf16)
            nc.sync.dma_start(wcomb_sb[:], wcomb_d[:].rearrange("(c p) n -> p c n", p=128))
            bcomb_sb = cp.tile([1, 512], bf16)
            nc.sync.dma_start(bcomb_sb[:], bcomb_d[:])
            whh_sb = cp.tile([128, 4, 1536], bf16)
            nc.sync.dma_start(whh_sb[:], whh_d[:].rearrange("(c p) n -> p c n", p=128))
            wihA_sb = cp.tile([128, 4, 1536], bf16)
            nc.sync.dma_start(wihA_sb[:], wihA_d[:].rearrange("(c p) n -> p c n", p=128))
            combo_sb = cp.tile([65, 1536], bf16)
            nc.sync.dma_start(combo_sb[:], combo_d[:])
            whead_sb = cp.tile([128, 4], bf16)
            nc.sync.dma_start(whead_sb[:], whead_d[:])
            bhead_sb = cp.tile([1, 1], f32)
            nc.sync.dma_start(bhead_sb[:], bhead_d[:])
            ones8 = cp.tile([1, 8], bf16)
            nc.vector.memset(ones8[:], 1.0)
            ones128 = cp.tile([128, 1], bf16)
            nc.vector.memset(ones128[:], 1.0)

            # ---- persistent state
            spb = []
            for b in range(BS):
                spb_t = st.tile([1, 512], bf16, tag=f"sp{b}", name=f"spb{b}")
                spb.append(spb_t)
            inp0T_bf = st.tile([128, 4, BS], bf16, tag="inp0b")  # ctx mean ^T
            nc.sync.dma_start(inp0T_bf[:],
                              inp0T_d[:].rearrange("p (c b) -> p c b", b=BS))
            h_sb = st.tile([BS, 512], f32, tag="h")            # h0 (batch-major)
            hT_b = st.tile([128, 4, BS], bf16, tag="hTb")      # h^T state (bf16)
            preds_sb = st.tile([1, T * BS], f32, tag="preds")  # pred_t batch-cols
            # combo lhsT [65, BS]: I8 rows 0-7, p+ row 32, p- row 64 (32-aligned
            # partition writes); matching zero rows in combo rhs make K=65 free.
            comboL = st.tile([65, BS], bf16, tag="comboL")
            nc.vector.memset(comboL[:], 0.0)
            nc.vector.tensor_copy(comboL[0:8, :], id_bf[0:8, 0:8])

            # ================= ATTENTION (software-pipelined) =================
            # Per batch b the work is split into:
            #   A(b):  xT DMA, qkT matmuls (+DVE bias-cast), then the scores
            #          stretch: score matmul + EXP per (h,qt), with the v
            #          matmuls and B2(b-1) interleaved to keep the PE busy
            #          while the ACT engine works through the EXPs.
            #   B1(b): m accumulation -> w=exp(m/HL) -> w'=w*u  (latency chain)
            #   B2(b): abar/abar^T/summary matmuls (throughput tail)
            # Emission: A(0); for b: B1(b-1), A(b){..B2(b-1)..}; B1(7); B2(7).
            with (
                tc.tile_pool(name="abuf", bufs=2) as ab,
                tc.tile_pool(name="awork", bufs=2) as aw,
                tc.tile_pool(name="pbig", bufs=3, space="PSUM") as pb,
                tc.tile_pool(name="psmall", bufs=1, space="PSUM") as psm,
                tc.tile_pool(name="pacc", bufs=2, space="PSUM") as pac,
            ):
                at = {}

                def stage_a(b, emit_b2):
                    xT = ab.tile([128, 4, 512], bf16, tag="xT", name=f"xT{b}")
                    nc.sync.dma_start(
                        xT[:],
                        ctxT_d[b * 512:(b + 1) * 512, :].rearrange(
                            "(c p) l -> p c l", p=128),
                    )
                    # q,k transposed: qkT[o_tile, l], 8 o-tiles (q:0-3, k:4-7)
                    qkT = ab.tile([128, 8, 512], bf16, tag="qkT",
                                  name=f"qkT{b}")
                    for ot in range(8):
                        ps = pb.tile([128, 512], f32, tag="mm",
                                     name=f"qk{b}_{ot}")
                        for ccc in range(4):
                            nc.tensor.matmul(
                                ps[:], wqk_sb[:, ccc, ot * 128:(ot + 1) * 128],
                                xT[:, ccc, :], start=(ccc == 0),
                                stop=(ccc == 3))
                        nc.vector.tensor_scalar(qkT[:, ot, :], ps[:],
                                                bqk_sb[:, ot:ot + 1], None,
                                                ALU.add)
                    E = ab.tile([128, 16, 512], bf16, tag="E", name=f"E{b}")
                    S_sb = aw.tile([128, 16], f32, tag="S", name=f"S{b}")
                    v_sb = ab.tile([128, 4, 512], bf16, tag="v", name=f"v{b}")
                    at[b] = dict(E=E, S=S_sb, v=v_sb)

                    # filler thunks for the scores stretch: v matmuls
                    fillers = []
                    vps = [None]

                    def v_mm(lt, ccc):
                        def go():
                            if ccc == 0:
                                vps[0] = pb.tile([128, 512], f32, tag="mm",
                                                 name=f"v{b}_{lt}")
                            nc.tensor.matmul(
                                vps[0][:], xT[:, ccc, lt * 128:(lt + 1) * 128],
                                wv_sb[:, ccc, :], start=(ccc == 0),
                                stop=(ccc == 3))
                            if ccc == 3:
                                nc.vector.tensor_copy(v_sb[:, lt, :], vps[0][:])
                        return go

                    for lt in range(4):
                        for ccc in range(4):
                            fillers.append(v_mm(lt, ccc))

                    # scores -> exp (+ row sums) per (head, q-tile)
                    for h in range(4):
                        for qt in range(4):
                            ps = pb.tile([128, 512], f32, tag="mm",
                                         name=f"sc{b}_{h}{qt}")
                            nc.tensor.matmul(
                                ps[:], qkT[:, h, qt * 128:(qt + 1) * 128],
                                qkT[:, 4 + h, :], start=True, stop=True)
                            hq = h * 4 + qt
                            nc.scalar.activation(
                                E[:, hq, :], ps[:], AF.Exp, scale=ISQD,
                                accum_out=S_sb[:, hq:hq + 1])
                            for _ in range(2):
                                if fillers:
                                    fillers.pop(0)()
                            if hq == 7 and emit_b2 is not None:
                                emit_b2()
                    while fillers:
                        fillers.pop(0)()

                def stage_b1(b):
                    E, S_sb = at[b]["E"], at[b]["S"]
                    u_f = aw.tile([128, 16], f32, tag="u", name=f"u{b}")
                    nc.vector.reciprocal(u_f[:], S_sb[:])
                    u_bf = aw.tile([128, 16], bf16, tag="ubf", name=f"ub{b}")
                    nc.vector.tensor_copy(u_bf[:], u_f[:])
                    # m[k] = sum_{h,q} E/S  (unnormalized attn-weight mean)
                    m_ps = pac.tile([1, 512], f32, tag="acc", name=f"m{b}")
                    for hq in range(16):
                        nc.tensor.matmul(
                            m_ps[:], u_bf[:, hq:hq + 1], E[:, hq, :],
                            start=(hq == 0), stop=(hq == 15))
                    m_sb = aw.tile([1, 512], f32, tag="msb", name=f"ms{b}")
                    nc.vector.tensor_copy(m_sb[:], m_ps[:])
                    # transpose m -> [512k, 1] as [128, 4]
                    mT_ps = psm.tile([128, 4], f32, tag="sm", name=f"mT{b}")
                    for cc in range(4):
                        nc.tensor.transpose(
                            mT_ps[:, cc:cc + 1],
                            m_sb[0:1, cc * 128:(cc + 1) * 128],
                            id_f32[0:1, 0:1])
                    # w = exp(m/2048) (normalization folded into rZ)
                    ew_f = aw.tile([128, 4], f32, tag="ewf", name=f"ew{b}")
                    nc.scalar.activation(ew_f[:], mT_ps[:], AF.Exp, scale=IHL)
                    ew_bf = aw.tile([128, 4], bf16, tag="ewbf", name=f"ewb{b}")
                    nc.vector.tensor_copy(ew_bf[:], ew_f[:])
                    zs_ps = psm.tile([1, 4], f32, tag="sm", name=f"zs{b}")
                    nc.tensor.matmul(zs_ps[:], ones128[:], ew_bf[:],
                                     start=True, stop=True)
                    z_sb = aw.tile([1, 1], f32, tag="zsb", name=f"z{b}")
                    nc.vector.reduce_sum(z_sb[:], zs_ps[:], axis=AX.X)
                    rz_sb = aw.tile([1, 1], f32, tag="rzsb", name=f"rz{b}")
                    nc.vector.reciprocal(rz_sb[:], z_sb[:])
                    # w' = w * u per head  [128, (h,cc)]
                    wp_f = aw.tile([128, 16], f32, tag="wpf", name=f"wp{b}")
                    for h in range(4):
                        nc.vector.tensor_tensor(
                            wp_f[:, h * 4:(h + 1) * 4], ew_f[:],
                            u_f[:, h * 4:(h + 1) * 4], ALU.mult)
                    wp_bf = aw.tile([128, 16], bf16, tag="wpbf", name=f"wpb{b}")
                    nc.vector.tensor_copy(wp_bf[:], wp_f[:])
                    at[b].update(wp=wp_bf, rz=rz_sb)

                def stage_b2(b):
                    E, v_sb = at[b]["E"], at[b]["v"]
                    wp_bf, rz_sb = at[b]["wp"], at[b]["rz"]
                    # abar_h[j] = sum_q w'_hq E[h,q,j]; rows at 32*h
                    ab_ps = pac.tile([128, 512], f32, tag="acc2",
                                     name=f"ab{b}")
                    # cc-outer so the four col-group strips stream
                    # concurrently (pc-monotone starts would serialize the
                    # h-outer order)
                    for cc in range(4):
                        for h in range(4):
                            nc.tensor.matmul(
                                ab_ps[h * 32:h * 32 + 1, :],
                                wp_bf[:, h * 4 + cc:h * 4 + cc + 1],
                                E[:, h * 4 + cc, :],
                                start=(cc == 0), stop=(cc == 3),
                                tile_position=(0, h * 32))
                    # abar rows -> 4 partition-0 tiles (engine writes must be
                    # 32-aligned in partition), then transpose each chunk
                    abh = []
                    for h in range(4):
                        abh_t = aw.tile([1, 512], f32, tag=f"ab{h}",
                                        name=f"abh{h}_{b}")
                        abh.append(abh_t)
                    for h in range(4):
                        nc.vector.tensor_copy(
                            abh[h][:], ab_ps[h * 32:h * 32 + 1, :])
                    # abar^T: [j, h] as [128, cc, h] (f32: PSUM 4B alignment)
                    abT_ps = psm.tile([128, 4, 4], f32, tag="sm",
                                      name=f"abT{b}")
                    for h in range(4):
                        for cc in range(4):
                            nc.tensor.transpose(
                                abT_ps[:, cc, h:h + 1],
                                abh[h][0:1, cc * 128:(cc + 1) * 128],
                                id_f32[0:1, 0:1])
                    abT_sb = aw.tile([128, 4, 4], bf16, tag="abTsb",
                                     name=f"abTs{b}")
                    nc.vector.tensor_copy(abT_sb[:], abT_ps[:])
                    # summary_pre[h*128+d] = sum_j abar_h[j] v[j, h*128+d]
                    # col-packed: head h accumulates at partition 32h so the
                    # four j-chunk rounds stream concurrently
                    sum_ps = pac.tile([97, 512], f32, tag="acc", name=f"su{b}")
                    for cc in range(4):
                        for h in range(4):
                            nc.tensor.matmul(
                                sum_ps[32 * h:32 * h + 1,
                                       h * 128:(h + 1) * 128],
                                abT_sb[:, cc, h:h + 1],
                                v_sb[:, cc, h * 128:(h + 1) * 128],
                                start=(cc == 0), stop=(cc == 3),
                                tile_position=(0, 32 * h))
                    # collect with 1/Z normalization (per-b partition-0 tile)
                    for h in range(4):
                        nc.vector.tensor_copy(
                            spb[b][0:1, h * 128:(h + 1) * 128],
                            sum_ps[32 * h:32 * h + 1, h * 128:(h + 1) * 128])
                    nc.vector.tensor_scalar(
                        spb[b][:], spb[b][:], rz_sb[0:1, 0:1], None, ALU.mult)

                stage_a(0, None)
                for b in range(1, BS):
                    stage_b1(b - 1)
                    stage_a(b, (lambda bb: (lambda: stage_b2(bb)))(b - 1))
                stage_b1(BS - 1)
                stage_b2(BS - 1)

                # ---- h0 = sp @ Wcomb + bcomb (batch-major out)
                # assemble [8, 512] via SBUF->SBUF DMA (DMA may write any row)
                sp_bf = aw.tile([BS, 512], bf16, tag="spbf")
                for b in range(BS):
                    nc.sync.dma_start(sp_bf[b:b + 1, :], spb[b][:])
                spT_ps = psm.tile([128, 4, BS], bf16, tag="sm")
                for cc in range(4):
                    nc.tensor.transpose(
                        spT_ps[:, cc, :], sp_bf[0:BS, cc * 128:(cc + 1) * 128],
                        id_bf[0:BS, 0:BS])
                spT_sb = aw.tile([128, 4, BS], bf16, tag="spTsb")
                nc.vector.tensor_copy(spT_sb[:], spT_ps[:])
                h0_ps = pac.tile([BS, 512], f32, tag="acc")
                for cc in range(4):
                    nc.tensor.matmul(h0_ps[:], spT_sb[:, cc, :],
                                     wcomb_sb[:, cc, :],
                                     start=(cc == 0), stop=False)
                nc.tensor.matmul(h0_ps[:], ones8[:], bcomb_sb[:],
                                 start=False, stop=True)
                nc.vector.tensor_copy(h_sb[:], h0_ps[:])

            # ======================== GRU scan (v2) ========================
            with (
                tc.tile_pool(name="gwork", bufs=2) as gw,
                tc.tile_pool(name="pgate", bufs=2, space="PSUM") as pg,
                tc.tile_pool(name="ptr", bufs=1, space="PSUM") as ptr,
            ):
                # ---- seed transposed h state from h0
                hbf = gw.tile([BS, 512], bf16, tag="hbf")
                nc.vector.tensor_copy(hbf[:], h_sb[:])
                hT0_ps = ptr.tile([128, 4, BS], bf16, tag="hT0")
                for cc in range(4):
                    nc.tensor.transpose(
                        hT0_ps[:, cc, :], hbf[0:BS, cc * 128:(cc + 1) * 128],
                        id_bf[0:BS, 0:BS])
                nc.vector.tensor_copy(hT_b[:], hT0_ps[:])

                preds_view = preds_sb[0:1, :].rearrange("p (b t) -> p b t", t=T)

                def pred_block(t_out, update_combo):
                    """pred_{t_out} from current hT_b; optionally update p+/-."""
                    pT_ps = ptr.tile([1, BS], f32, tag="pT", name=f"pT{t_out}")
                    for cc in range(4):
                        nc.tensor.matmul(pT_ps[:], whead_sb[:, cc:cc + 1],
                                         hT_b[:, cc, :],
                                         start=(cc == 0), stop=(cc == 3))
                    nc.scalar.activation(
                        preds_view[:, :, t_out], pT_ps[:],
                        AF.Identity, bias=bhead_sb[0:1, 0:1])
                    if update_combo:
                        nc.vector.tensor_scalar_max(comboL[32:33, :], pT_ps[:], 0.0)
                        nc.vector.tensor_scalar_min(comboL[64:65, :], pT_ps[:], 0.0)

                def dummy_warm(k, late=None):
                    """PE warm-keepers: k small (N=64) matmuls into a
                    never-read PSUM tile.  Each blocks the in-order tensor
                    queue for only ~70ns, but together they keep the HAM
                    activity window busy through the serial ACT/DVE
                    stretches so the PE clock holds at 2.4GHz.  The WAW
                    chain on the shared tile plus program order pins them
                    to this step's idle window."""
                    dum = ptr.tile([BS, 512], f32, tag="dum", name="dum")
                    for j in range(k):
                        if late is not None:
                            nc.tensor.matmul(dum[0:BS, :],
                                             late[:, 0:BS], late[:, :],
                                             start=(j == 0),
                                             stop=(j == k - 1),
                                             tile_position=(96, 0))
                        else:
                            nc.tensor.matmul(dum[0:BS, :],
                                             hT_b[:, j % 4, :],
                                             whh_sb[:, j % 4, 0:512],
                                             start=(j == 0),
                                             stop=(j == k - 1),
                                             tile_position=(0, 0))

                for t in range(1, T + 1):
                    # pred_{t-2} = h_{t-1} @ W_head; its p+/- rows feed this
                    # step's combo matmuls (x_t's inp part).
                    if t >= 2:
                        pred_block(t - 2, update_combo=True)

                    # ---- gate matmuls: one PSUM bank, 4 column groups
                    g_ps = pg.tile([128, 512], f32, tag="g")
                    rP = g_ps[0:BS, :]
                    zP = g_ps[32:32 + BS, :]
                    hP = g_ps[64:64 + BS, :]
                    iP = g_ps[96:96 + BS, :]
                    # r/z/hn h-projections interleaved across col groups 0/1/2
                    for cc in range(4):
                        nc.tensor.matmul(rP, hT_b[:, cc, :],
                                         whh_sb[:, cc, 0:512],
                                         start=(cc == 0), stop=False,
                                         tile_position=(0, 0))
                        nc.tensor.matmul(zP, hT_b[:, cc, :],
                                         whh_sb[:, cc, 512:1024],
                                         start=(cc == 0), stop=False,
                                         tile_position=(0, 32))
                        nc.tensor.matmul(hP, hT_b[:, cc, :],
                                         whh_sb[:, cc, 1024:1536],
                                         start=(cc == 0), stop=(cc == 3),
                                         tile_position=(0, 64))
                    if t == 1:
                        for cc in range(4):
                            nc.tensor.matmul(rP, inp0T_bf[:, cc, :],
                                             wihA_sb[:, cc, 0:512],
                                             start=False, stop=False,
                                             tile_position=(0, 0))
                            nc.tensor.matmul(zP, inp0T_bf[:, cc, :],
                                             wihA_sb[:, cc, 512:1024],
                                             start=False, stop=False,
                                             tile_position=(0, 32))
                    # combo contributions last (they wait on this step's p+/-;
                    # the whh rounds above cover that latency)
                    nc.tensor.matmul(rP, comboL[:], combo_sb[:, 0:512],
                                     start=False, stop=True,
                                     tile_position=(0, 0))
                    nc.tensor.matmul(zP, comboL[:], combo_sb[:, 512:1024],
                                     start=False, stop=True,
                                     tile_position=(0, 32))
                    # i_n (combo only) in col group 3; group stays open for
                    # the t-accumulate matmul below.
                    nc.tensor.matmul(iP, comboL[:], combo_sb[:, 1024:1536],
                                     start=True, stop=False,
                                     tile_position=(0, 96))
                    if t == 1:  # x_1 inp-part: real matmul with context mean
                        for cc in range(4):
                            nc.tensor.matmul(
                                iP, inp0T_bf[:, cc, :],
                                wihA_sb[:, cc, 1024:1536],
                                start=False, stop=False, tile_position=(0, 96))

                    # ---- gate math.  sigmoid(z) is emitted AFTER t so the
                    # shared ACT semaphore doesn't make t wait for it.
                    rs_sb = gw.tile([BS, 512], bf16, tag="rs")
                    nc.scalar.activation(rs_sb[0:BS, :], rP, AF.Sigmoid)
                    # t = r * h_n  (the one batch-major DVE op)
                    t_sb = gw.tile([BS, 512], bf16, tag="t")
                    nc.vector.tensor_tensor(t_sb[0:BS, :], rs_sb[0:BS, :], hP,
                                            ALU.mult)
                    zs_sb = gw.tile([40, 512], bf16, tag="zs")
                    nc.scalar.activation(zs_sb[32:32 + BS, :], zP, AF.Sigmoid)
                    # i_n += t on the tensor engine (identity matmul closes
                    # the i_n accumulation group)
                    nc.tensor.matmul(iP, id_bf[0:BS, 0:BS], t_sb[0:BS, :],
                                     start=False, stop=True,
                                     tile_position=(0, 96))
                    # z transposes + warm-keepers run during tanh
                    zT_ps = ptr.tile([128, 4, BS], bf16, tag="zT",
                                     name=f"zT{t}")
                    for cc in range(4):
                        nc.tensor.transpose(
                            zT_ps[:, cc, :],
                            zs_sb[32:32 + BS, cc * 128:(cc + 1) * 128],
                            id_bf[32:40, 32:40], tile_position=(32, 0))
                    n_sb = gw.tile([104, 512], bf16, tag="n")
                    nc.scalar.activation(n_sb[96:96 + BS, :], iP, AF.Tanh)

                    nT_ps = ptr.tile([128, 4, BS], bf16, tag="nT",
                                     name=f"nT{t}")
                    for cc in range(4):
                        nc.tensor.transpose(
                            nT_ps[:, cc, :],
                            n_sb[96:96 + BS, cc * 128:(cc + 1) * 128],
                            id_bf[96:104, 96:104], tile_position=(96, 0))

                    # ---- h update in transposed layout (FD=32 on 128 parts)
                    dT = gw.tile([128, 4, BS], f32, tag="dT")
                    nc.vector.tensor_tensor(dT[:], hT_b[:], nT_ps[:],
                                            ALU.subtract)
                    eT = gw.tile([128, 4, BS], f32, tag="eT")
                    nc.vector.tensor_tensor(eT[:], dT[:], zT_ps[:], ALU.mult)
                    nc.vector.tensor_tensor(hT_b[:], nT_ps[:], eT[:], ALU.add)

                # epilogue: final pred (uses h_T)
                pred_block(T - 1, update_combo=False)

                # write out preds (sbuf already [b, t]-ordered, contiguous)
                nc.sync.dma_start(preds_d[:, :], preds_sb[0:1, :])

    nc.compile()
    return nc


# ---------------------------------------------------------------- host glue
def _prep_core_inputs(inputs, core):
    import ml_dtypes

    bf16 = ml_dtypes.bfloat16
    f32 = np.float32
    ctx = np.asarray(inputs["context"], f32)
    se = np.asarray(inputs["static_embed"], f32)
    W_in = np.asarray(inputs["W_in_attn"], f32)
    b_in = np.asarray(inputs["b_in_attn"], f32)
    W_out = np.asarray(inputs["W_out_attn"], f32)
    b_out = np.asarray(inputs["b_out_attn"], f32)
    W_init = np.asarray(inputs["W_init"], f32)
    b_init = np.asarray(inputs["b_init"], f32)
    W_ih = np.asarray(inputs["W_ih"], f32)
    b_ih = np.asarray(inputs["b_ih"], f32)
    W_hh = np.asarray(inputs["W_hh"], f32)
    b_hh = np.asarray(inputs["b_hh"], f32)
    W_tf = np.asarray(inputs["W_tf"], f32)
    W_head = np.asarray(inputs["W_head"], f32)
    b_head = np.asarray(inputs["b_head"], f32)

    sl = slice(core * BS, (core + 1) * BS)
    ctxT = np.ascontiguousarray(ctx[sl].transpose(0, 2, 1)).reshape(BS * 512, 512)
    # context mean (GRU step-1 input), transposed to [128, (cc, b)]
    inp0 = ctx[sl].mean(axis=1)                          # [BS, 512]
    inp0T = np.ascontiguousarray(
        inp0.T.reshape(4, 128, BS).transpose(1, 0, 2)).reshape(128, 4 * BS)

    # attention weights
    wqk = np.ascontiguousarray(W_in[:1024].T)            # [512, 1024]
    wv = np.ascontiguousarray(W_in[1024:].T)             # [512, 512]
    bqk = np.ascontiguousarray(b_in[:1024].reshape(8, 128).T)  # [128, 8]
    # summary -> h0 composition (v-bias folds in via sum(w)=1)
    b_out_eff = b_in[1024:] @ W_out.T + b_out
    wcomb = np.ascontiguousarray(W_out.T @ W_init.T)     # [512, 512]
    bcomb = (b_out_eff @ W_init.T + b_init).reshape(1, 512)

    # GRU weights
    whh = np.ascontiguousarray(W_hh.T)                   # [512, 1536]
    W_ihA = W_ih[:, :512]
    W_ihB = W_ih[:, 512:]
    wihA = np.ascontiguousarray(W_ihA.T)                 # [512, 1536]
    u = W_tf[:, 0]
    a_plus = np.maximum(u, 0.0) @ W_ihA.T                # [1536]
    a_minus = (-np.maximum(-u, 0.0)) @ W_ihA.T           # [1536]
    combo = np.zeros((65, 1536), f32)
    combo[0:8] = se[sl] @ W_ihB.T + b_ih                 # static + b_ih
    combo[0:8, 0:1024] += b_hh[0:1024]                   # b_hh folds into r,z
    combo[32] = a_plus
    combo[64] = a_minus
    whead = np.ascontiguousarray(W_head[0].reshape(4, 128).T)  # [128, 4]
    bhead = np.asarray(b_head, f32).reshape(1, 1)

    to_bf = lambda x: np.ascontiguousarray(x).astype(bf16)
    return {
        "ctxT": to_bf(ctxT), "inp0T": to_bf(inp0T), "wqk": to_bf(wqk),
        "wv": to_bf(wv),
        "bqk": np.ascontiguousarray(bqk, f32), "wcomb": to_bf(wcomb),
        "bcomb": to_bf(bcomb), "whh": to_bf(whh), "wihA": to_bf(wihA),
        "combo": to_bf(combo), "whead": to_bf(whead), "bhead": bhead,
    }


def _run_bass(inputs, trace=False):
    global _COMPILED
    from concourse.bass_utils import run_bass_kernel_spmd

    if _COMPILED is None:
        _COMPILED = _build()
    nc = _COMPILED
    in_maps = [_prep_core_inputs(inputs, c) for c in range(NCORES)]
    res = run_bass_kernel_spmd(nc, in_maps, list(range(NCORES)), trace=trace)
    preds = np.concatenate([res.results[c]["preds"] for c in range(NCORES)],
                           axis=0)  # [64, 64]
    return preds.astype(np.float32).reshape(B, T, 1), res


def kernel(**inputs) -> np.ndarray:
    fs = int(np.asarray(inputs["future_steps"]))
    b_tf = np.asarray(inputs["b_tf"], np.float32)
    b_hh = np.asarray(inputs["b_hh"], np.float32)
    shapes_ok = (
        fs == T
        and np.asarray(inputs["context"]).shape == (B, LC, C)
        and np.asarray(inputs["static_embed"]).shape == (B, S)
        and not np.any(b_tf)
        and not np.any(b_hh[1024:])
    )
    if not shapes_ok:
        print("kernel.py: unexpected shapes/biases; using numpy fallback",
              file=sys.stderr)
        return _np_fallback(inputs)
    try:
        out, _ = _run_bass(inputs, trace=False)
        return out
    except Exception:
        traceback.print_exc()
        print("kernel.py: bass path failed; using numpy fallback",
              file=sys.stderr)
        return _np_fallback(inputs)


# revision 37
# speedup vs baseline: 1.0321x; 1.0321x over previous
"""nn_ARDecoder Trainium2 Bass kernel.

Shapes (hardcoded): context [64,512,512] f32, static_embed [64,128] f32,
H=4 heads, future_steps=64, OUT=1. Output preds [64,64,1] f32.

Sharding: data-parallel over batch B=64 across 8 cores (8 batches/core),
weights replicated, GRU scan local per shard. No collectives.

Algebraic structure exploited:
  - reference only uses sum_q w_q * attn_out[b,q,:], so the full a@v einsum
    and out-projection collapse to tiny vector contractions; W_out and
    W_init compose into one matrix host-side.
  - OUT=1 makes W_tf rank-1: with b_tf==0, inp_t = relu(pred*u) =
    p+ * relu(u) + p- * (-relu(-u)), so the inp part of the GRU input
    projection is rank-2 (two precomputed 1536-vectors scaled by
    max(pred,0)/min(pred,0) per batch).
  - static_embed part of gi is step-invariant: precomputed host-side.

GRU scan performance structure (v2):
  - The three h-projection gates (r, z, hn) and the i_n combo run in four
    PE column groups concurrently (tile_position=(0,32j)), all into one
    PSUM bank at partition quadrants 0/32/64/96.  Matmul stream time per
    step ~= 5 x 213ns instead of 15 x 213ns.
  - sigmoid/tanh evacuate PSUM->SBUF as part of the activation (no
    separate copies).  t = r*h_n is the only batch-major DVE op; the
    "+ i_n" add is done on the tensor engine by accumulating t into the
    i_n PSUM group via an identity matmul.
  - The h update runs in transposed layout [512dims x 8batch] on 128
    partitions (DVE free-dim 32 instead of 512), and h stays transposed
    across steps, feeding the next step's matmuls directly as lhsT.
"""

import os
import sys
import traceback

import numpy as np

sys.path.insert(0, "/opt/trn_rl_repo")

H = 4
B, LC, C, S = 64, 512, 512, 128
T = 64  # future steps
NCORES = 8
BS = B // NCORES  # 8 batches per core

_COMPILED = None  # (nc,) cache


# ---------------------------------------------------------------- numpy ref
def _np_softmax(x, axis):
    m = np.max(x, axis=axis, keepdims=True)
    e = np.exp(x - m)
    return e / np.sum(e, axis=axis, keepdims=True)


def _np_sigmoid(x):
    out = np.empty_like(x)
    pos = x >= 0
    out[pos] = 1.0 / (1.0 + np.exp(-x[pos]))
    ex = np.exp(x[~pos])
    out[~pos] = ex / (1.0 + ex)
    return out


def _np_fallback(inputs):
    """Exact numpy replica of the reference (correctness fallback)."""
    context = np.asarray(inputs["context"], np.float32)
    static_embed = np.asarray(inputs["static_embed"], np.float32)
    W_in, b_in = inputs["W_in_attn"], inputs["b_in_attn"]
    W_out, b_out = inputs["W_out_attn"], inputs["b_out_attn"]
    W_init, b_init = inputs["W_init"], inputs["b_init"]
    W_ih, b_ih = inputs["W_ih"], inputs["b_ih"]
    W_hh, b_hh = inputs["W_hh"], inputs["b_hh"]
    W_tf, b_tf = inputs["W_tf"], inputs["b_tf"]
    W_head, b_head = inputs["W_head"], inputs["b_head"]
    fs = int(np.asarray(inputs["future_steps"]))

    Bn, L, Cn = context.shape
    D = Cn // H
    qkv = context @ W_in.T + b_in
    q, k, v = np.split(qkv, 3, axis=-1)
    heads = lambda t: t.reshape(Bn, L, H, D).transpose(0, 2, 1, 3)
    q, k, v = heads(q), heads(k), heads(v)
    scores = np.einsum("bhqd,bhkd->bhqk", q, k) / np.sqrt(np.float32(D))
    a = _np_softmax(scores, axis=-1)
    out = np.einsum("bhqk,bhkd->bhqd", a, v).transpose(0, 2, 1, 3).reshape(Bn, L, Cn)
    attn_out = out @ W_out.T + b_out
    attn_w = a.mean(axis=1)
    w = _np_softmax(attn_w.mean(axis=1), axis=-1)
    summary = np.einsum("bk,bkc->bc", w, attn_out)
    h = summary @ W_init.T + b_init
    inp = context.mean(axis=1)
    preds = np.empty((Bn, fs, W_head.shape[0]), np.float32)
    for t in range(fs):
        x = np.concatenate([inp, static_embed], axis=-1)
        gi = x @ W_ih.T + b_ih
        gh = h @ W_hh.T + b_hh
        i_r, i_z, i_n = np.split(gi, 3, axis=-1)
        h_r, h_z, h_n = np.split(gh, 3, axis=-1)
        r = _np_sigmoid(i_r + h_r)
        z = _np_sigmoid(i_z + h_z)
        n = np.tanh(i_n + r * h_n)
        h = (1.0 - z) * n + z * h
        pred = h @ W_head.T + b_head
        inp = np.maximum(pred @ W_tf.T + b_tf, 0.0)
        preds[:, t, :] = pred
    return preds


# ---------------------------------------------------------------- builder
def _build():
    import concourse.bacc as bacc
    import concourse.mybir as mybir
    import concourse.tile as tile
    from concourse import masks

    dt = mybir.dt
    AF = mybir.ActivationFunctionType
    ALU = mybir.AluOpType
    AX = mybir.AxisListType

    nc = bacc.Bacc("TRN2", target_bir_lowering=False, debug=False,
                   num_devices=NCORES)

    f32, bf16 = dt.float32, dt.bfloat16

    # DRAM I/O (per-core shard tensors; host preps layouts)
    ctxT_d = nc.dram_tensor("ctxT", [BS * 512, 512], bf16, kind="ExternalInput")
    inp0T_d = nc.dram_tensor("inp0T", [128, 4 * BS], bf16, kind="ExternalInput")
    wqk_d = nc.dram_tensor("wqk", [512, 1024], bf16, kind="ExternalInput")
    wv_d = nc.dram_tensor("wv", [512, 512], bf16, kind="ExternalInput")
    bqk_d = nc.dram_tensor("bqk", [128, 8], f32, kind="ExternalInput")
    wcomb_d = nc.dram_tensor("wcomb", [512, 512], bf16, kind="ExternalInput")
    bcomb_d = nc.dram_tensor("bcomb", [1, 512], bf16, kind="ExternalInput")
    whh_d = nc.dram_tensor("whh", [512, 1536], bf16, kind="ExternalInput")
    wihA_d = nc.dram_tensor("wihA", [512, 1536], bf16, kind="ExternalInput")
    combo_d = nc.dram_tensor("combo", [65, 1536], bf16, kind="ExternalInput")
    whead_d = nc.dram_tensor("whead", [128, 4], bf16, kind="ExternalInput")
    bhead_d = nc.dram_tensor("bhead", [1, 1], f32, kind="ExternalInput")
    preds_d = nc.dram_tensor("preds", [BS, T], f32, kind="ExternalOutput")

    ISQD = 1.0 / np.sqrt(128.0)  # 1/sqrt(D)
    IHL = 1.0 / (H * 512.0)      # 1/(H*L) for attn-weight mean
    with tile.TileContext(nc) as tc:
        with (
            tc.tile_pool(name="const", bufs=1) as cp,
            tc.tile_pool(name="state", bufs=1) as st,
        ):
            # ---- constants into SBUF
            id_bf = cp.tile([128, 128], bf16)
            masks.make_identity(nc, id_bf[:])
            id_f32 = cp.tile([128, 128], f32)
            masks.make_identity(nc, id_f32[:])
            wqk_sb = cp.tile([128, 4, 1024], bf16)
            nc.sync.dma_start(wqk_sb[:], wqk_d[:].rearrange("(c p) n -> p c n", p=128))
            wv_sb = cp.tile([128, 4, 512], bf16)
            nc.sync.dma_start(wv_sb[:], wv_d[:].rearrange("(c p) n -> p c n", p=128))
            bqk_sb = cp.tile([128, 8], f32)
            nc.sync.dma_start(bqk_sb[:], bqk_d[:])
            wcomb_sb = cp.tile([128, 4, 512], bf16)
            nc.sync.dma_start(wcomb_sb[:], wcomb_d[:].rearrange("(c p) n -> p c n", p=128))
            bcomb_sb = cp.tile([1, 512], bf16)
            nc.sync.dma_start(bcomb_sb[:], bcomb_d[:])
            whh_sb = cp.tile([128, 4, 1536], bf16)
            nc.sync.dma_start(whh_sb[:], whh_d[:].rearrange("(c p) n -> p c n", p=128))
            wihA_sb = cp.tile([128, 4, 1536], bf16)
            nc.sync.dma_start(wihA_sb[:], wihA_d[:].rearrange("(c p) n -> p c n", p=128))
            combo_sb = cp.tile([65, 1536], bf16)
            nc.sync.dma_start(combo_sb[:], combo_d[:])
            whead_sb = cp.tile([128, 4], bf16)
            nc.sync.dma_start(whead_sb[:], whead_d[:])
            bhead_sb = cp.tile([1, 1], f32)
            nc.sync.dma_start(bhead_sb[:], bhead_d[:])
            ones8 = cp.tile([1, 8], bf16)
            nc.vector.memset(ones8[:], 1.0)
            ones128 = cp.tile([128, 1], bf16)
            nc.vector.memset(ones128[:], 1.0)

            # ---- persistent state
            spb = []
            for b in range(BS):
                spb_t = st.tile([1, 512], bf16, tag=f"sp{b}", name=f"spb{b}")
                spb.append(spb_t)
            inp0T_bf = st.tile([128, 4, BS], bf16, tag="inp0b")  # ctx mean ^T
            nc.sync.dma_start(inp0T_bf[:],
                              inp0T_d[:].rearrange("p (c b) -> p c b", b=BS))
            h_sb = st.tile([BS, 512], f32, tag="h")            # h0 (batch-major)
            hT_b = st.tile([128, 4, BS], bf16, tag="hTb")      # h^T state (bf16)
            preds_sb = st.tile([1, T * BS], f32, tag="preds")  # pred_t batch-cols
            # combo lhsT [65, BS]: I8 rows 0-7, p+ row 32, p- row 64 (32-aligned
            # partition writes); matching zero rows in combo rhs make K=65 free.
            comboL = st.tile([65, BS], bf16, tag="comboL")
            nc.vector.memset(comboL[:], 0.0)
            nc.vector.tensor_copy(comboL[0:8, :], id_bf[0:8, 0:8])

            # ================= ATTENTION (software-pipelined) =================
            # Per batch b the work is split into:
            #   A(b):  xT DMA, qkT matmuls (+DVE bias-cast), then the scores
            #          stretch: score matmul + EXP per (h,qt), with the v
            #          matmuls and B2(b-1) interleaved to keep the PE busy
            #          while the ACT engine works through the EXPs.
            #   B1(b): m accumulation -> w=exp(m/HL) -> w'=w*u  (latency chain)
            #   B2(b): abar/abar^T/summary matmuls (throughput tail)
            # Emission: A(0); for b: B1(b-1), A(b){..B2(b-1)..}; B1(7); B2(7).
            with (
                tc.tile_pool(name="abuf", bufs=2) as ab,
                tc.tile_pool(name="awork", bufs=2) as aw,
                tc.tile_pool(name="pbig", bufs=3, space="PSUM") as pb,
                tc.tile_pool(name="psmall", bufs=1, space="PSUM") as psm,
                tc.tile_pool(name="pacc", bufs=2, space="PSUM") as pac,
            ):
                at = {}

                def stage_a(b, emit_b2):
                    xT = ab.tile([128, 4, 512], bf16, tag="xT", name=f"xT{b}")
                    nc.sync.dma_start(
                        xT[:],
                        ctxT_d[b * 512:(b + 1) * 512, :].rearrange(
                            "(c p) l -> p c l", p=128),
                    )
                    # q,k transposed: qkT[o_tile, l], 8 o-tiles (q:0-3, k:4-7)
                    qkT = ab.tile([128, 8, 512], bf16, tag="qkT",
                                  name=f"qkT{b}")
                    for ot in range(8):
                        ps = pb.tile([128, 512], f32, tag="mm",
                                     name=f"qk{b}_{ot}")
                        for ccc in range(4):
                            nc.tensor.matmul(
                                ps[:], wqk_sb[:, ccc, ot * 128:(ot + 1) * 128],
                                xT[:, ccc, :], start=(ccc == 0),
                                stop=(ccc == 3))
                        nc.vector.tensor_scalar(qkT[:, ot, :], ps[:],
                                                bqk_sb[:, ot:ot + 1], None,
                                                ALU.add)
                    E = ab.tile([128, 16, 512], bf16, tag="E", name=f"E{b}")
                    S_sb = aw.tile([128, 16], f32, tag="S", name=f"S{b}")
                    v_sb = ab.tile([128, 4, 512], bf16, tag="v", name=f"v{b}")
                    at[b] = dict(E=E, S=S_sb, v=v_sb)

                    # filler thunks for the scores stretch: v matmuls
                    fillers = []
                    vps = [None]

                    def v_mm(lt, ccc):
                        def go():
                            if ccc == 0:
                                vps[0] = pb.tile([128, 512], f32, tag="mm",
                                                 name=f"v{b}_{lt}")
                            nc.tensor.matmul(
                                vps[0][:], xT[:, ccc, lt * 128:(lt + 1) * 128],
                                wv_sb[:, ccc, :], start=(ccc == 0),
                                stop=(ccc == 3))
                            if ccc == 3:
                                nc.vector.tensor_copy(v_sb[:, lt, :], vps[0][:])
                        return go

                    for lt in range(4):
                        for ccc in range(4):
                            fillers.append(v_mm(lt, ccc))

                    # scores -> exp (+ row sums) per (head, q-tile)
                    for h in range(4):
                        for qt in range(4):
                            ps = pb.tile([128, 512], f32, tag="mm",
                                         name=f"sc{b}_{h}{qt}")
                            nc.tensor.matmul(
                                ps[:], qkT[:, h, qt * 128:(qt + 1) * 128],
                                qkT[:, 4 + h, :], start=True, stop=True)
                            hq = h * 4 + qt
                            nc.scalar.activation(
                                E[:, hq, :], ps[:], AF.Exp, scale=ISQD,
                                accum_out=S_sb[:, hq:hq + 1])
                            for _ in range(2):
                                if fillers:
                                    fillers.pop(0)()
                            if hq == 7 and emit_b2 is not None:
                                emit_b2()
                    while fillers:
                        fillers.pop(0)()

                def stage_b1(b):
                    E, S_sb = at[b]["E"], at[b]["S"]
                    u_f = aw.tile([128, 16], f32, tag="u", name=f"u{b}")
                    nc.vector.reciprocal(u_f[:], S_sb[:])
                    u_bf = aw.tile([128, 16], bf16, tag="ubf", name=f"ub{b}")
                    nc.vector.tensor_copy(u_bf[:], u_f[:])
                    # m[k] = sum_{h,q} E/S  (unnormalized attn-weight mean)
                    m_ps = pac.tile([1, 512], f32, tag="acc", name=f"m{b}")
                    for hq in range(16):
                        nc.tensor.matmul(
                            m_ps[:], u_bf[:, hq:hq + 1], E[:, hq, :],
                            start=(hq == 0), stop=(hq == 15))
                    m_sb = aw.tile([1, 512], f32, tag="msb", name=f"ms{b}")
                    nc.vector.tensor_copy(m_sb[:], m_ps[:])
                    # transpose m -> [512k, 1] as [128, 4]
                    mT_ps = psm.tile([128, 4], f32, tag="sm", name=f"mT{b}")
                    for cc in range(4):
                        nc.tensor.transpose(
                            mT_ps[:, cc:cc + 1],
                            m_sb[0:1, cc * 128:(cc + 1) * 128],
                            id_f32[0:1, 0:1])
                    # w = exp(m/2048) (normalization folded into rZ)
                    ew_f = aw.tile([128, 4], f32, tag="ewf", name=f"ew{b}")
                    nc.scalar.activation(ew_f[:], mT_ps[:], AF.Exp, scale=IHL)
                    ew_bf = aw.tile([128, 4], bf16, tag="ewbf", name=f"ewb{b}")
                    nc.vector.tensor_copy(ew_bf[:], ew_f[:])
                    zs_ps = psm.tile([1, 4], f32, tag="sm", name=f"zs{b}")
                    nc.tensor.matmul(zs_ps[:], ones128[:], ew_bf[:],
                                     start=True, stop=True)
                    z_sb = aw.tile([1, 1], f32, tag="zsb", name=f"z{b}")
                    nc.vector.reduce_sum(z_sb[:], zs_ps[:], axis=AX.X)
                    rz_sb = aw.tile([1, 1], f32, tag="rzsb", name=f"rz{b}")
                    nc.vector.reciprocal(rz_sb[:], z_sb[:])
                    # w' = w * u per head  [128, (h,cc)]
                    wp_f = aw.tile([128, 16], f32, tag="wpf", name=f"wp{b}")
                    for h in range(4):
                        nc.vector.tensor_tensor(
                            wp_f[:, h * 4:(h + 1) * 4], ew_f[:],
                            u_f[:, h * 4:(h + 1) * 4], ALU.mult)
                    wp_bf = aw.tile([128, 16], bf16, tag="wpbf", name=f"wpb{b}")
                    nc.vector.tensor_copy(wp_bf[:], wp_f[:])
                    at[b].update(wp=wp_bf, rz=rz_sb)

                def stage_b2(b):
                    E, v_sb = at[b]["E"], at[b]["v"]
                    wp_bf, rz_sb = at[b]["wp"], at[b]["rz"]
                    # abar_h[j] = sum_q w'_hq E[h,q,j]; rows at 32*h
                    ab_ps = pac.tile([128, 512], f32, tag="acc2",
                                     name=f"ab{b}")
                    # cc-outer so the four col-group strips stream
                    # concurrently (pc-monotone starts would serialize the
                    # h-outer order)
                    for cc in range(4):
                        for h in range(4):
                            nc.tensor.matmul(
                                ab_ps[h * 32:h * 32 + 1, :],
                                wp_bf[:, h * 4 + cc:h * 4 + cc + 1],
                                E[:, h * 4 + cc, :],
                                start=(cc == 0), stop=(cc == 3),
                                tile_position=(0, h * 32))
                    # abar rows -> 4 partition-0 tiles (engine writes must be
                    # 32-aligned in partition), then transpose each chunk
                    abh = []
                    for h in range(4):
                        abh_t = aw.tile([1, 512], f32, tag=f"ab{h}",
                                        name=f"abh{h}_{b}")
                        abh.append(abh_t)
                    for h in range(4):
                        nc.vector.tensor_copy(
                            abh[h][:], ab_ps[h * 32:h * 32 + 1, :])
                    # abar^T: [j, h] as [128, cc, h] (f32: PSUM 4B alignment)
                    abT_ps = psm.tile([128, 4, 4], f32, tag="sm",
                                      name=f"abT{b}")
                    for h in range(4):
                        for cc in range(4):
                            nc.tensor.transpose(
                                abT_ps[:, cc, h:h + 1],
                                abh[h][0:1, cc * 128:(cc + 1) * 128],
                                id_f32[0:1, 0:1])
                    abT_sb = aw.tile([128, 4, 4], bf16, tag="abTsb",
                                     name=f"abTs{b}")
                    nc.vector.tensor_copy(abT_sb[:], abT_ps[:])
                    # summary_pre[h*128+d] = sum_j abar_h[j] v[j, h*128+d]
                    # col-packed: head h accumulates at partition 32h so the
                    # four j-chunk rounds stream concurrently
                    sum_ps = pac.tile([97, 512], f32, tag="acc", name=f"su{b}")
                    for cc in range(4):
                        for h in range(4):
                            nc.tensor.matmul(
                                sum_ps[32 * h:32 * h + 1,
                                       h * 128:(h + 1) * 128],
                                abT_sb[:, cc, h:h + 1],
                                v_sb[:, cc, h * 128:(h + 1) * 128],
                                start=(cc == 0), stop=(cc == 3),
                                tile_position=(0, 32 * h))
                    # collect with 1/Z normalization (per-b partition-0 tile)
                    for h in range(4):
                        nc.vector.tensor_copy(
                            spb[b][0:1, h * 128:(h + 1) * 128],
                            sum_ps[32 * h:32 * h + 1, h * 128:(h + 1) * 128])
                    nc.vector.tensor_scalar(
                        spb[b][:], spb[b][:], rz_sb[0:1, 0:1], None, ALU.mult)

                stage_a(0, None)
                for b in range(1, BS):
                    stage_b1(b - 1)
                    stage_a(b, (lambda bb: (lambda: stage_b2(bb)))(b - 1))
                stage_b1(BS - 1)
                stage_b2(BS - 1)

                # ---- h0 = sp @ Wcomb + bcomb (batch-major out)
                # assemble [8, 512] via SBUF->SBUF DMA (DMA may write any row)
                sp_bf = aw.tile([BS, 512], bf16, tag="spbf")
                for b in range(BS):
                    nc.sync.dma_start(sp_bf[b:b + 1, :], spb[b][:])
                spT_ps = psm.tile([128, 4, BS], bf16, tag="sm")
                for cc in range(4):
                    nc.tensor.transpose(
                        spT_ps[:, cc, :], sp_bf[0:BS, cc * 128:(cc + 1) * 128],
                        id_bf[0:BS, 0:BS])
                spT_sb = aw.tile([128, 4, BS], bf16, tag="spTsb")
                nc.vector.tensor_copy(spT_sb[:], spT_ps[:])
                h0_ps = pac.tile([BS, 512], f32, tag="acc")
                for cc in range(4):
                    nc.tensor.matmul(h0_ps[:], spT_sb[:, cc, :],
                                     wcomb_sb[:, cc, :],
                                     start=(cc == 0), stop=False)
                nc.tensor.matmul(h0_ps[:], ones8[:], bcomb_sb[:],
                                 start=False, stop=True)
                nc.vector.tensor_copy(h_sb[:], h0_ps[:])

            # ======================== GRU scan (v2) ========================
            with (
                tc.tile_pool(name="gwork", bufs=2) as gw,
                tc.tile_pool(name="pgate", bufs=2, space="PSUM") as pg,
                tc.tile_pool(name="ptr", bufs=1, space="PSUM") as ptr,
            ):
                # ---- seed transposed h state from h0
                hbf = gw.tile([BS, 512], bf16, tag="hbf")
                nc.vector.tensor_copy(hbf[:], h_sb[:])
                hT0_ps = ptr.tile([128, 4, BS], bf16, tag="hT0")
                for cc in range(4):
                    nc.tensor.transpose(
                        hT0_ps[:, cc, :], hbf[0:BS, cc * 128:(cc + 1) * 128],
                        id_bf[0:BS, 0:BS])
                nc.vector.tensor_copy(hT_b[:], hT0_ps[:])

                preds_view = preds_sb[0:1, :].rearrange("p (b t) -> p b t", t=T)

                def pred_block(t_out, update_combo):
                    """pred_{t_out} from current hT_b; optionally update p+/-."""
                    pT_ps = ptr.tile([1, BS], f32, tag="pT", name=f"pT{t_out}")
                    for cc in range(4):
                        nc.tensor.matmul(pT_ps[:], whead_sb[:, cc:cc + 1],
                                         hT_b[:, cc, :],
                                         start=(cc == 0), stop=(cc == 3))
                    nc.scalar.activation(
                        preds_view[:, :, t_out], pT_ps[:],
                        AF.Identity, bias=bhead_sb[0:1, 0:1])
                    if update_combo:
                        nc.vector.tensor_scalar_max(comboL[32:33, :], pT_ps[:], 0.0)
                        nc.vector.tensor_scalar_min(comboL[64:65, :], pT_ps[:], 0.0)

                def dummy_warm(k, late=None):
                    """PE warm-keepers: k small (N=64) matmuls into a
                    never-read PSUM tile.  Each blocks the in-order tensor
                    queue for only ~70ns, but together they keep the HAM
                    activity window busy through the serial ACT/DVE
                    stretches so the PE clock holds at 2.4GHz.  The WAW
                    chain on the shared tile plus program order pins them
                    to this step's idle window."""
                    dum = ptr.tile([BS, 512], f32, tag="dum", name="dum")
                    for j in range(k):
                        if late is not None:
                            nc.tensor.matmul(dum[0:BS, :],
                                             late[:, 0:BS], late[:, :],
                                             start=(j == 0),
                                             stop=(j == k - 1),
                                             tile_position=(96, 0))
                        else:
                            nc.tensor.matmul(dum[0:BS, :],
                                             hT_b[:, j % 4, :],
                                             whh_sb[:, j % 4, 0:512],
                                             start=(j == 0),
                                             stop=(j == k - 1),
                                             tile_position=(0, 0))

                for t in range(1, T + 1):
                    # pred_{t-2} = h_{t-1} @ W_head; its p+/- rows feed this
                    # step's combo matmuls (x_t's inp part).
                    if t >= 2:
                        pred_block(t - 2, update_combo=True)

                    # ---- gate matmuls: one PSUM bank, 4 column groups
                    g_ps = pg.tile([128, 512], f32, tag="g")
                    rP = g_ps[0:BS, :]
                    zP = g_ps[32:32 + BS, :]
                    hP = g_ps[64:64 + BS, :]
                    iP = g_ps[96:96 + BS, :]
                    # r/z/hn h-projections interleaved across col groups 0/1/2
                    for cc in range(4):
                        nc.tensor.matmul(rP, hT_b[:, cc, :],
                                         whh_sb[:, cc, 0:512],
                                         start=(cc == 0), stop=False,
                                         tile_position=(0, 0))
                        nc.tensor.matmul(zP, hT_b[:, cc, :],
                                         whh_sb[:, cc, 512:1024],
                                         start=(cc == 0), stop=False,
                                         tile_position=(0, 32))
                        nc.tensor.matmul(hP, hT_b[:, cc, :],
                                         whh_sb[:, cc, 1024:1536],
                                         start=(cc == 0), stop=(cc == 3),
                                         tile_position=(0, 64))
                    if t == 1:
                        for cc in range(4):
                            nc.tensor.matmul(rP, inp0T_bf[:, cc, :],
                                             wihA_sb[:, cc, 0:512],
                                             start=False, stop=False,
                                             tile_position=(0, 0))
                            nc.tensor.matmul(zP, inp0T_bf[:, cc, :],
                                             wihA_sb[:, cc, 512:1024],
                                             start=False, stop=False,
                                             tile_position=(0, 32))
                    # combo contributions last (they wait on this step's p+/-;
                    # the whh rounds above cover that latency)
                    nc.tensor.matmul(rP, comboL[:], combo_sb[:, 0:512],
                                     start=False, stop=True,
                                     tile_position=(0, 0))
                    nc.tensor.matmul(zP, comboL[:], combo_sb[:, 512:1024],
                                     start=False, stop=True,
                                     tile_position=(0, 32))
                    # i_n (combo only) in col group 3; group stays open for
                    # the t-accumulate matmul below.
                    nc.tensor.matmul(iP, comboL[:], combo_sb[:, 1024:1536],
                                     start=True, stop=False,
                                     tile_position=(0, 96))
                    if t == 1:  # x_1 inp-part: real matmul with context mean
                        for cc in range(4):
                            nc.tensor.matmul(
                                iP, inp0T_bf[:, cc, :],
                                wihA_sb[:, cc, 1024:1536],
                                start=False, stop=False, tile_position=(0, 96))
                    dummy_warm(3)  # PE busy during sigmoid(r) / t

                    # ---- gate math.  sigmoid(z) is emitted AFTER t so the
                    # shared ACT semaphore doesn't make t wait for it.
                    rs_sb = gw.tile([BS, 512], bf16, tag="rs")
                    nc.scalar.activation(rs_sb[0:BS, :], rP, AF.Sigmoid)
                    # t = r * h_n  (the one batch-major DVE op)
                    t_sb = gw.tile([BS, 512], bf16, tag="t")
                    nc.vector.tensor_tensor(t_sb[0:BS, :], rs_sb[0:BS, :], hP,
                                            ALU.mult)
                    zs_sb = gw.tile([40, 512], bf16, tag="zs")
                    nc.scalar.activation(zs_sb[32:32 + BS, :], zP, AF.Sigmoid)
                    # i_n += t on the tensor engine (identity matmul closes
                    # the i_n accumulation group)
                    nc.tensor.matmul(iP, id_bf[0:BS, 0:BS], t_sb[0:BS, :],
                                     start=False, stop=True,
                                     tile_position=(0, 96))
                    # z transposes + warm-keepers run during tanh
                    zT_ps = ptr.tile([128, 4, BS], bf16, tag="zT",
                                     name=f"zT{t}")
                    for cc in range(4):
                        nc.tensor.transpose(
                            zT_ps[:, cc, :],
                            zs_sb[32:32 + BS, cc * 128:(cc + 1) * 128],
                            id_bf[32:40, 32:40], tile_position=(32, 0))
                    dummy_warm(2)
                    n_sb = gw.tile([104, 512], bf16, tag="n")
                    nc.scalar.activation(n_sb[96:96 + BS, :], iP, AF.Tanh)

                    nT_ps = ptr.tile([128, 4, BS], bf16, tag="nT",
                                     name=f"nT{t}")
                    for cc in range(4):
                        nc.tensor.transpose(
                            nT_ps[:, cc, :],
                            n_sb[96:96 + BS, cc * 128:(cc + 1) * 128],
                            id_bf[96:104, 96:104], tile_position=(96, 0))
                    # PE busy during the h-update chain
                    dummy_warm(3, late=n_sb[96:96 + BS, :])

                    # ---- h update in transposed layout (FD=32 on 128 parts)
                    dT = gw.tile([128, 4, BS], f32, tag="dT")
                    nc.vector.tensor_tensor(dT[:], hT_b[:], nT_ps[:],
                                            ALU.subtract)
                    eT = gw.tile([128, 4, BS], f32, tag="eT")
                    nc.vector.tensor_tensor(eT[:], dT[:], zT_ps[:], ALU.mult)
                    nc.vector.tensor_tensor(hT_b[:], nT_ps[:], eT[:], ALU.add)

                # epilogue: final pred (uses h_T)
                pred_block(T - 1, update_combo=False)

                # write out preds (sbuf already [b, t]-ordered, contiguous)
                nc.sync.dma_start(preds_d[:, :], preds_sb[0:1, :])

    nc.compile()
    return nc


# ---------------------------------------------------------------- host glue
def _prep_core_inputs(inputs, core):
    import ml_dtypes

    bf16 = ml_dtypes.bfloat16
    f32 = np.float32
    ctx = np.asarray(inputs["context"], f32)
    se = np.asarray(inputs["static_embed"], f32)
    W_in = np.asarray(inputs["W_in_attn"], f32)
    b_in = np.asarray(inputs["b_in_attn"], f32)
    W_out = np.asarray(inputs["W_out_attn"], f32)
    b_out = np.asarray(inputs["b_out_attn"], f32)
    W_init = np.asarray(inputs["W_init"], f32)
    b_init = np.asarray(inputs["b_init"], f32)
    W_ih = np.asarray(inputs["W_ih"], f32)
    b_ih = np.asarray(inputs["b_ih"], f32)
    W_hh = np.asarray(inputs["W_hh"], f32)
    b_hh = np.asarray(inputs["b_hh"], f32)
    W_tf = np.asarray(inputs["W_tf"], f32)
    W_head = np.asarray(inputs["W_head"], f32)
    b_head = np.asarray(inputs["b_head"], f32)

    sl = slice(core * BS, (core + 1) * BS)
    ctxT = np.ascontiguousarray(ctx[sl].transpose(0, 2, 1)).reshape(BS * 512, 512)
    # context mean (GRU step-1 input), transposed to [128, (cc, b)]
    inp0 = ctx[sl].mean(axis=1)                          # [BS, 512]
    inp0T = np.ascontiguousarray(
        inp0.T.reshape(4, 128, BS).transpose(1, 0, 2)).reshape(128, 4 * BS)

    # attention weights
    wqk = np.ascontiguousarray(W_in[:1024].T)            # [512, 1024]
    wv = np.ascontiguousarray(W_in[1024:].T)             # [512, 512]
    bqk = np.ascontiguousarray(b_in[:1024].reshape(8, 128).T)  # [128, 8]
    # summary -> h0 composition (v-bias folds in via sum(w)=1)
    b_out_eff = b_in[1024:] @ W_out.T + b_out
    wcomb = np.ascontiguousarray(W_out.T @ W_init.T)     # [512, 512]
    bcomb = (b_out_eff @ W_init.T + b_init).reshape(1, 512)

    # GRU weights
    whh = np.ascontiguousarray(W_hh.T)                   # [512, 1536]
    W_ihA = W_ih[:, :512]
    W_ihB = W_ih[:, 512:]
    wihA = np.ascontiguousarray(W_ihA.T)                 # [512, 1536]
    u = W_tf[:, 0]
    a_plus = np.maximum(u, 0.0) @ W_ihA.T                # [1536]
    a_minus = (-np.maximum(-u, 0.0)) @ W_ihA.T           # [1536]
    combo = np.zeros((65, 1536), f32)
    combo[0:8] = se[sl] @ W_ihB.T + b_ih                 # static + b_ih
    combo[0:8, 0:1024] += b_hh[0:1024]                   # b_hh folds into r,z
    combo[32] = a_plus
    combo[64] = a_minus
    whead = np.ascontiguousarray(W_head[0].reshape(4, 128).T)  # [128, 4]
    bhead = np.asarray(b_head, f32).reshape(1, 1)

    to_bf = lambda x: np.ascontiguousarray(x).astype(bf16)
    return {
        "ctxT": to_bf(ctxT), "inp0T": to_bf(inp0T), "wqk": to_bf(wqk),
        "wv": to_bf(wv),
        "bqk": np.ascontiguousarray(bqk, f32), "wcomb": to_bf(wcomb),
        "bcomb": to_bf(bcomb), "whh": to_bf(whh), "wihA": to_bf(wihA),
        "combo": to_bf(combo), "whead": to_bf(whead), "bhead": bhead,
    }


def _run_bass(inputs, trace=False):
    global _COMPILED
    from concourse.bass_utils import run_bass_kernel_spmd

    if _COMPILED is None:
        _COMPILED = _build()
    nc = _COMPILED
    in_maps = [_prep_core_inputs(inputs, c) for c in range(NCORES)]
    res = run_bass_kernel_spmd(nc, in_maps, list(range(NCORES)), trace=trace)
    preds = np.concatenate([res.results[c]["preds"] for c in range(NCORES)],
                           axis=0)  # [64, 64]
    return preds.astype(np.float32).reshape(B, T, 1), res


def kernel(**inputs) -> np.ndarray:
    fs = int(np.asarray(inputs["future_steps"]))
    b_tf = np.asarray(inputs["b_tf"], np.float32)
    b_hh = np.asarray(inputs["b_hh"], np.float32)
    shapes_ok = (
        fs == T
        and np.asarray(inputs["context"]).shape == (B, LC, C)
        and np.asarray(inputs["static_embed"]).shape == (B, S)
        and not np.any(b_tf)
        and not np.any(b_hh[1024:])
    )
    if not shapes_ok:
        print("kernel.py: unexpected shapes/biases; using numpy fallback",
              file=sys.stderr)
        return _np_fallback(inputs)
    try:
        out, _ = _run_bass(inputs, trace=False)
        return out
    except Exception:
        traceback.print_exc()
        print("kernel.py: bass path failed; using numpy fallback",
              file=sys.stderr)
        return _np_fallback(inputs)
